# revision 72
# baseline (speedup 1.0000x reference)
"""OTAM / DSN_TEMPORAL meta-logits kernel for 8 Trainium2 NeuronCores.

Strategy (data-parallel over queries, per sharding hint):
  - 2048 queries sharded 256/core across 8 cores; support prototypes replicated.
  - bf16 datapath: inputs cast to bf16 on the host; PSUM accumulation fp32.
  - Frame cosine similarities via PE matmuls.
  - Query norms via PE gram matmuls (lhsT = rhs = query chunk) + diagonal
    extraction with a masked scalar_tensor_tensor accum (one [128,1] column per
    (l,g) group, already in the per-partition layout the EXP scale needs) +
    rsqrt on ACT as exp(-0.5*ln(nsq)).  This removes the old DVE squares,
    ones-matmul chains, SBUF-DMA transposes and Newton iterations from the
    critical DVE stream.
  - Core reformulation: with E = exp(-cum/lambda), lambda=0.5, the OTAM
    soft-min DP becomes a pure multiply-add recurrence
        E[l][m] = ed[l][m] * (E_diag + E_left + mask*E_up),  ed = exp(-2*d)
    with no transcendentals in the serial chain.
  - The DP runs wavefront-style over anti-diagonals on the DVE in bf16, with
    the per-(l,g) gram diag extraction interleaved between diagonals so the
    wavefront can start as soon as ed row 0 exists and track the matmuls.
  - Both DP orientations live in one unified E grid; the second orientation
    reads ed with transposed cell indices (no transposed copy).
  - The zero-pad last column (ed=1) has closed form 2*sum(col8) - last.
  - Tail fuses ln(E9A)+ln(E9B) into one ln(E9A*E9B).

kernel() accepts FULL inputs and returns the FULL [2048, 64] logits.
"""

import numpy as np

# ---- problem constants (hardcoded per contest contract) ----
NCORES = 8
NQ_TOT = 2048          # total queries
NQ = NQ_TOT // NCORES  # queries per core = 256
L = 8                  # query frames
S = 8                  # support frames
D = 576                # feature dim
DPAD = 640             # padded feature dim (5 * 128)
KC = 5                 # K chunks of 128
NS = 64                # support classes
G = 2                  # query groups of 128 instances (NQ = 256 = 2*128)
NSF = NS * S           # 512 support frames
LG = L * G             # 16 (l, g) matmul groups per core
LN2 = 0.6931471805599453

# E-grid cell layout: 256 lanes = [dir(2), g(2), sj(64)]; row stride 8 cells.
# ed keeps 128-lane cells [g(2), sj(64)]; dir2 reads it with transposed cell
# indices (edT[r][scol] == ed[scol][r]), so no transposed copy is materialized.
CELL = 256
ROW = 8 * CELL         # 2048
DIAG = ROW - CELL      # 1792: (r+1, w-1) step
ECELL = 128
EROW = 8 * ECELL       # 1024
ED1 = EROW - ECELL     # 896: ed (r+1, scol-1) step for dir1
ED2 = ECELL - EROW     # dir2 reads (scol, r): step along diag = -896


def _build_program():
    import concourse.bass as bass
    import concourse.bacc as bacc
    import concourse.mybir as mybir
    import concourse.tile as tile
    from contextlib import ExitStack

    dt = mybir.dt.float32
    db = mybir.dt.bfloat16
    d8 = mybir.dt.float8e4
    OP = mybir.AluOpType
    ACTF = mybir.ActivationFunctionType

    nc = bacc.Bacc("TRN2", target_bir_lowering=False, debug=False, num_devices=NCORES)

    qd = nc.dram_tensor("qslabs", [L * 128, KC * 256], d8, kind="ExternalInput")
    sd = nc.dram_tensor("sfeat8", [128, KC * NSF], d8, kind="ExternalInput")
    md = nc.dram_tensor("imask", [128, 128], db, kind="ExternalInput")
    od = nc.dram_tensor("logits", [NQ, NS], dt, kind="ExternalOutput")

    def V(t, off, dims, nparts=None):
        ap = t[:]
        p0 = list(ap.ap[0]) if nparts is None else [list(ap.ap[0])[0], nparts]
        return bass.AP(ap.tensor, ap.offset + off, [p0] + [list(d) for d in dims])

    def cE(r, w):          # E grid cell offset
        return r * ROW + w * CELL

    def cD(r, scol):       # ed grid cell offset (dir1 view)
        return r * EROW + scol * ECELL

    with tile.TileContext(nc) as tc:
        with ExitStack() as ctx:
            const = ctx.enter_context(tc.tile_pool(name="const", bufs=1))
            big = ctx.enter_context(tc.tile_pool(name="big", bufs=1))
            slabs = ctx.enter_context(tc.tile_pool(name="slabs", bufs=8))
            scr = ctx.enter_context(tc.tile_pool(name="scr", bufs=2))
            psm = ctx.enter_context(tc.tile_pool(name="psm", bufs=2, space="PSUM"))
            psg = ctx.enter_context(tc.tile_pool(name="psg", bufs=3, space="PSUM"))
            psn = ctx.enter_context(tc.tile_pool(name="psn", bufs=1, space="PSUM"))
            pss = ctx.enter_context(tc.tile_pool(name="pss", bufs=1, space="PSUM"))

            ones_nk = const.tile([128, 1], db)   # lhsT for norm matmuls (K=128, M=1)
            ones_b = const.tile([1, 128], db)    # lhsT for broadcast matmul (K=1, M=128)
            neg2 = const.tile([128, 1], dt)      # ACT bias for exp(2cos - 2)
            ln2 = const.tile([128, 1], dt)       # ACT bias ln(2) for 2/sn
            imask = const.tile([128, 128], db)   # identity mask for gram diag
            nc.vector.memset(ones_nk[:], 1.0)
            nc.vector.memset(ones_b[:], 1.0)
            nc.vector.memset(neg2[:], -2.0)
            nc.vector.memset(ln2[:], LN2)

            # Preload the one ACT table that covers Ln+Exp+Copy so the
            # fixpoint pass never inserts per-transition table reloads
            # (act_func_set_id 6 = natural_log_exp_and_others).
            nc.scalar.add_instruction(mybir.InstLoadActFuncSet(
                name=nc.scalar.bass.get_next_instruction_name(),
                act_func_set_id=6, ins=[], outs=[]))

            s8 = big.tile([128, KC, NSF], d8)            # fp8 raw support
            sT = big.tile([128, KC, NSF], db)            # bf16 scaled support
            ed = big.tile([128, 8 * EROW], db)           # 16KB/p
            E = big.tile([128, 8 * ROW], db)             # 32KB/p DP grid
            nsq = big.tile([128, LG], dt)                # col = l*2+g (|q|^2)
            rqn = big.tile([128, LG], dt)                # 1/|q|
            lnq = big.tile([128, LG], dt)                # ln(nsq) staging
            snsq = big.tile([128, 4], dt)                # |s_f|^2, partition=frame
            slnq = big.tile([128, 4], dt)
            rsn = big.tile([128, 4], db)                 # 2/|s_f| bf16
            rsnT = big.tile([1, 512], db)                # transposed row for bcast
            pbs = big.tile([128, NSF], db)               # bf16 copy of 2/sn bcast
            fin = big.tile([128, 2816], db)              # final-reduction scratch
            junk = big.tile([128, 128], db)              # stt masked-out scratch
            e9m = big.tile([128, 128], dt)               # E9A*E9B fp32
            outf = big.tile([128, 128], dt)              # fp32 logits staging

            # ---------- DMAs upfront, all on sync in dependency order:
            # support + mask first (gate the whole chain), then slabs in l
            # order so descriptors drain in consumption order.
            nc.sync.dma_start(V(s8, 0, [[1, KC * NSF]]), sd.ap())
            nc.sync.dma_start(imask[:], md.ap())
            stiles = {}
            for l in range(L):
                slab = slabs.tile([128, KC, 256], d8)
                stiles[l] = slab
                nc.sync.dma_start(V(slab, 0, [[1, KC * 256]]),
                                  qd.ap()[l * 128:(l + 1) * 128, :])

            # ---------- support prep: fp8 gram norms -> 2/sn -> scale ----------
            # High priority: this chain gates mains(0).  Support norms come
            # from fp8 gram diag blocks (like the query norms) so no squares
            # or [1,512] ln/exp sit on the critical path.
            with tc.high_priority():
                sg = pss.tile([128, 4, 128], dt)
                for b in range(4):
                    for k in range(KC):
                        sb = s8[:, k, b * 128:(b + 1) * 128]
                        nc.tensor.matmul(sg[:, b], sb, sb,
                                         start=(k == 0), stop=(k == KC - 1))
                for b in range(4):
                    nc.vector.scalar_tensor_tensor(
                        junk[:], sg[:, b], 1.0, imask[:],
                        OP.bypass, OP.mult, accum_out=snsq[:, b:b + 1])
                # rsn = 2/|s_f| = exp(-0.5*ln(|s|^2) + ln2), partition=frame
                nc.scalar.activation(slnq[:], snsq[:], ACTF.Ln)
                nc.scalar.activation(rsn[:], slnq[:], ACTF.Exp, bias=ln2[:],
                                     scale=-0.5)
                # transpose each [128,1] column to a [1,128] partition-0 row
                # (PE), gather to one [1,512] row, broadcast into pb via one
                # K=1 matmul, and scale s8 -> bf16 sT in one pass
                pt = psn.tile([1, 4, 128], db, tag="pt")
                for b in range(4):
                    nc.tensor.transpose(pt[:, b], rsn[:, b:b + 1], imask[:])
                nc.vector.tensor_copy(rsnT[:], V(pt, 0, [[1, 512]]))
                pb = psn.tile([128, NSF], dt, tag="pb")
                nc.tensor.matmul(pb[:], ones_b[:], rsnT[:], start=True, stop=True)
                nc.scalar.copy(pbs[:], pb[:])
                for k in range(KC):
                    nc.vector.tensor_tensor(sT[:, k, :], s8[:, k, :], pbs[:],
                                            OP.mult)

            # ---------- query norms: gram matmul + masked diag extract ----------
            gps = {}

            def gram(l):
                gp = psg.tile([128, G, 128], dt, tag="gp")
                gps[l] = gp
                for g in range(G):
                    for k in range(KC):
                        q = stiles[l][:, k, g * 128:(g + 1) * 128]
                        nc.tensor.matmul(gp[:, g], q, q,
                                         start=(k == 0), stop=(k == KC - 1))

            def stt_diag(l):
                # accum_out = sum over free of (gram * I) = |q|^2 per partition
                for g in range(G):
                    lg = l * G + g
                    nc.vector.scalar_tensor_tensor(
                        junk[:], gps[l][:, g], 1.0, imask[:],
                        OP.bypass, OP.mult, accum_out=nsq[:, lg:lg + 1])

            def rqn_batch(lo, hi):
                # rqn = 1/|q| = exp(-0.5*ln(nsq)) on ACT, for lg in [lo, hi)
                nc.scalar.activation(lnq[:, lo:hi], nsq[:, lo:hi], ACTF.Ln)
                nc.scalar.activation(rqn[:, lo:hi], lnq[:, lo:hi], ACTF.Exp,
                                     scale=-0.5)

            def mains(l, split=False):
                # split=True: two N=256 halves with separate exps, so the
                # first ed frames of the row land ~1us earlier (matters only
                # for the last rows, which gate the wavefront tail)
                for g in range(G):
                    lg = l * G + g
                    pm = psm.tile([128, NSF], dt, tag="mm")
                    halves = ((0, 256), (256, 512)) if split else ((0, 512),)
                    for lo, hi in halves:
                        for k in range(KC):
                            nc.tensor.matmul(pm[:, lo:hi],
                                             stiles[l][:, k, g * 128:(g + 1) * 128],
                                             sT[:, k, lo:hi],
                                             start=(k == 0), stop=(k == KC - 1))
                        nsc = (hi - lo) // 64
                        edv = V(ed, cD(l, lo // 64) + g * 64,
                                [[ECELL, nsc], [1, 64]])
                        nc.scalar.activation(
                            edv,
                            pm[:, lo:hi].rearrange("p (s j) -> p s j", s=nsc),
                            ACTF.Exp, bias=neg2[:], scale=rqn[:, lg:lg + 1])

            # All grams + diag extracts + rqn batches run in the head (slabs
            # land well before the scaled support is ready), so the mains own
            # the PE window exclusively and the c-loop is pure wavefront.
            for l in range(L):
                gram(l)
                stt_diag(l)
                rqn_batch(2 * l, 2 * l + 2)
            mains(0)
            mains(1)

            # ---------- DP in L-shells (DVE-only, bf16) ----------
            # E cell (r, w) lanes: [dir*128 + g*64 + sj]; dir1 reads ed[r][w],
            # dir2 reads ed[w][r], so a unified cell needs ed rows <= max(r,w)
            # only.  Shell k = row-k strip + col-k strip runs right after
            # exp(k): the post-ed7 critical path is just shell 7, not the last
            # seven anti-diagonals.
            def edm(r, w):
                # both dirs' ed for cell (r, w) as one [2, 128] AP
                return V(ed, cD(r, w), [[cD(w, r) - cD(r, w), 2], [1, 128]])

            for k in range(8):
                if k >= 2:
                    mains(k)
                if k == 0:
                    for d in range(2):
                        nc.vector.tensor_copy(
                            V(E, cE(0, 0) + d * 128, [[1, 128]]),
                            V(ed, cD(0, 0), [[1, 128]]))
                    continue
                # col strip: (0, k) row-0 mult, then (1..k-1, k) batched
                nc.vector.tensor_tensor(
                    V(E, cE(0, k), [[1, 256]]),
                    V(E, cE(0, k - 1), [[1, 256]]), edm(0, k), OP.mult)
                n = k - 1
                if n >= 1:
                    nc.vector.tensor_tensor(
                        V(E, cE(1, k), [[ROW, n], [1, 256]]),
                        V(E, cE(0, k - 1), [[ROW, n], [1, 256]]),
                        V(E, cE(1, k - 1), [[ROW, n], [1, 256]]), OP.add)
                    out = V(E, cE(1, k), [[ROW, n], [1, 128]])
                    nc.vector.tensor_tensor(
                        out, out, V(ed, cD(1, k), [[EROW, n], [1, 64 * 2]]),
                        OP.mult)
                    out = V(E, cE(1, k) + 128, [[ROW, n], [1, 128]])
                    nc.vector.tensor_tensor(
                        out, out, V(ed, cD(k, 1), [[ECELL, n], [1, 128]]),
                        OP.mult)
                # row strip: (k, 0) boundary, then (k, 1..k) serial
                nc.vector.scalar_tensor_tensor(
                    V(E, cE(k, 0), [[1, 256]]),
                    V(E, cE(k - 1, 0), [[1, 256]]), 2.0,
                    edm(k, 0), OP.add, OP.mult)
                for w in range(1, k + 1):
                    nc.vector.tensor_tensor(
                        V(E, cE(k, w), [[1, 256]]),
                        V(E, cE(k - 1, w - 1), [[1, 256]]),
                        V(E, cE(k, w - 1), [[1, 256]]), OP.add)
                    if w == k:
                        # both dirs read the same ed cell; a 2-dim AP would
                        # need stride 0, so mult the two halves separately
                        for d in range(2):
                            out = V(E, cE(k, k) + d * 128, [[1, 128]])
                            nc.vector.tensor_tensor(
                                out, out, V(ed, cD(k, k), [[1, 128]]), OP.mult)
                    else:
                        nc.vector.tensor_tensor(
                            V(E, cE(k, w), [[1, 256]]),
                            V(E, cE(k, w), [[1, 256]]), edm(k, w), OP.mult)
                # col-7 pair reductions as their rows complete
                if k == 7:
                    for j in range(3):
                        nc.vector.tensor_tensor(
                            V(fin, j * CELL, [[1, CELL]]),
                            V(E, cE(2 * j, 7), [[1, CELL]]),
                            V(E, cE(2 * j + 1, 7), [[1, CELL]]), OP.add)
            nc.vector.tensor_tensor(
                V(fin, 3 * CELL, [[1, CELL]]),
                V(E, cE(6, 7), [[1, CELL]]),
                V(E, cE(7, 7), [[1, CELL]]), OP.add)

            # ---------- final pad-column closed form + logits ----------
            # E9 = 2*sum_r E[r][7] - E[7][7], then logits = 0.5*ln(E9A*E9B)
            nc.vector.tensor_tensor(
                V(fin, 2048, [[CELL, 2], [1, CELL]]),
                V(fin, 0, [[2 * CELL, 2], [1, CELL]]),
                V(fin, CELL, [[2 * CELL, 2], [1, CELL]]), OP.add)
            nc.vector.tensor_tensor(
                V(fin, 1024, [[1, CELL]]),
                V(fin, 2048, [[1, CELL]]),
                V(fin, 2048 + CELL, [[1, CELL]]), OP.add)
            e9 = V(fin, 1280, [[1, CELL]])
            nc.vector.scalar_tensor_tensor(
                e9, V(fin, 1024, [[1, CELL]]), 2.0,
                V(E, cE(7, 7), [[1, CELL]]), OP.mult, OP.subtract)
            # e9m = E9A * E9B (fp32), logits = 0.5*ln(e9m)
            nc.vector.tensor_tensor(e9m[:], V(fin, 1280, [[1, 128]]),
                                    V(fin, 1280 + 128, [[1, 128]]), OP.mult)
            outv = outf[:, 0:128]
            nc.scalar.activation(outv, e9m[:], ACTF.Ln)
            nc.vector.tensor_scalar_mul(outv, outv, 0.5)
            # DMA out: logits[q = g*128 + p, sj];  src free f = g*64 + sj
            oap = od.ap()
            dst = bass.AP(oap.tensor, oap.offset, [[NS, 128], [128 * NS, G], [1, NS]])
            nc.sync.dma_start(dst, outv)

    nc.compile()
    return nc


_CACHED = None


def _get_program():
    global _CACHED
    if _CACHED is None:
        _CACHED = _build_program()
    return _CACHED


def _prep_inputs(support_features, query_features):
    """Host-side data movement: shard queries, pad D to 640, reorder layouts."""
    import ml_dtypes
    bf16 = ml_dtypes.bfloat16
    e4m3 = ml_dtypes.float8_e4m3fn
    q = np.ascontiguousarray(query_features, dtype=np.float32)
    s = np.ascontiguousarray(support_features, dtype=np.float32)
    qp = np.zeros((NQ_TOT, L, DPAD), np.float32)
    qp[:, :, :D] = q
    sp = np.zeros((NSF, DPAD), np.float32)
    sp[:, :D] = s.reshape(NSF, D)
    # support frame reorder: scol = s*64 + sj  <->  frame sj*8 + s
    idx = (np.arange(NSF) % NS) * S + (np.arange(NSF) // NS)
    spr = sp[idx]                                   # [512, 640]
    sT_r = np.ascontiguousarray(
        spr.reshape(NSF, KC, 128).transpose(2, 1, 0)).reshape(128, KC * NSF).astype(e4m3)
    im = np.eye(128, dtype=bf16)
    in_maps = []
    for cidx in range(NCORES):
        qs = qp[cidx * NQ:(cidx + 1) * NQ]          # [256, 8, 640]
        q5 = qs.reshape(G, 128, L, KC, 128)          # [g, qi, l, k, dp]
        # slab for l: SBUF [128 part=dp, k, (g,qi)] -> host rows (l, dp), cols (k, g, qi)
        qT_r = np.ascontiguousarray(q5.transpose(2, 4, 3, 0, 1))  # [l, dp, k, g, qi]
        in_maps.append({
            "qslabs": qT_r.reshape(L * 128, KC * 256).astype(e4m3),
            "sfeat8": sT_r,
            "imask": im,
        })
    return in_maps


def kernel(support_features, query_features):
    from concourse.bass_utils import run_bass_kernel_spmd
    nc = _get_program()
    in_maps = _prep_inputs(support_features, query_features)
    res = run_bass_kernel_spmd(nc, in_maps, list(range(NCORES)))
    out = np.concatenate([res.results[i]["logits"] for i in range(NCORES)], axis=0)
    return out.astype(np.float32)


# revision 75
# speedup vs baseline: 1.1778x; 1.1778x over previous
"""OTAM / DSN_TEMPORAL meta-logits kernel for 8 Trainium2 NeuronCores.

Strategy (data-parallel over queries, per sharding hint):
  - 2048 queries sharded 256/core across 8 cores; support prototypes replicated.
  - bf16 datapath: inputs cast to bf16 on the host; PSUM accumulation fp32.
  - Frame cosine similarities via PE matmuls.
  - Query norms via PE gram matmuls (lhsT = rhs = query chunk) + diagonal
    extraction with a masked scalar_tensor_tensor accum (one [128,1] column per
    (l,g) group, already in the per-partition layout the EXP scale needs) +
    rsqrt on ACT as exp(-0.5*ln(nsq)).  This removes the old DVE squares,
    ones-matmul chains, SBUF-DMA transposes and Newton iterations from the
    critical DVE stream.
  - Core reformulation: with E = exp(-cum/lambda), lambda=0.5, the OTAM
    soft-min DP becomes a pure multiply-add recurrence
        E[l][m] = ed[l][m] * (E_diag + E_left + mask*E_up),  ed = exp(-2*d)
    with no transcendentals in the serial chain.
  - The DP runs wavefront-style over anti-diagonals on the DVE in bf16, with
    the per-(l,g) gram diag extraction interleaved between diagonals so the
    wavefront can start as soon as ed row 0 exists and track the matmuls.
  - Both DP orientations live in one unified E grid; the second orientation
    reads ed with transposed cell indices (no transposed copy).
  - The zero-pad last column (ed=1) has closed form 2*sum(col8) - last.
  - Tail fuses ln(E9A)+ln(E9B) into one ln(E9A*E9B).

kernel() accepts FULL inputs and returns the FULL [2048, 64] logits.
"""

import numpy as np

# ---- problem constants (hardcoded per contest contract) ----
NCORES = 8
NQ_TOT = 2048          # total queries
NQ = NQ_TOT // NCORES  # queries per core = 256
L = 8                  # query frames
S = 8                  # support frames
D = 576                # feature dim
DPAD = 640             # padded feature dim (5 * 128)
KC = 5                 # K chunks of 128
NS = 64                # support classes
G = 2                  # query groups of 128 instances (NQ = 256 = 2*128)
NSF = NS * S           # 512 support frames
LG = L * G             # 16 (l, g) matmul groups per core
LN2 = 0.6931471805599453

# E-grid cell layout: 256 lanes = [dir(2), g(2), sj(64)]; row stride 8 cells.
# ed keeps 128-lane cells [g(2), sj(64)]; dir2 reads it with transposed cell
# indices (edT[r][scol] == ed[scol][r]), so no transposed copy is materialized.
CELL = 256
ROW = 8 * CELL         # 2048
DIAG = ROW - CELL      # 1792: (r+1, w-1) step
ECELL = 128
EROW = 8 * ECELL       # 1024
ED1 = EROW - ECELL     # 896: ed (r+1, scol-1) step for dir1
ED2 = ECELL - EROW     # dir2 reads (scol, r): step along diag = -896


def _build_program():
    import concourse.bass as bass
    import concourse.bacc as bacc
    import concourse.mybir as mybir
    import concourse.tile as tile
    from contextlib import ExitStack

    dt = mybir.dt.float32
    db = mybir.dt.bfloat16
    d8 = mybir.dt.float8e4
    OP = mybir.AluOpType
    ACTF = mybir.ActivationFunctionType

    nc = bacc.Bacc("TRN2", target_bir_lowering=False, debug=False, num_devices=NCORES)

    qd = nc.dram_tensor("qslabs", [L * 128, KC * 256], d8, kind="ExternalInput")
    sd = nc.dram_tensor("sfeat8", [128, KC * NSF], d8, kind="ExternalInput")
    md = nc.dram_tensor("imask", [128, 128], db, kind="ExternalInput")
    od = nc.dram_tensor("logits", [NQ, NS], dt, kind="ExternalOutput")

    def V(t, off, dims, nparts=None):
        ap = t[:]
        p0 = list(ap.ap[0]) if nparts is None else [list(ap.ap[0])[0], nparts]
        return bass.AP(ap.tensor, ap.offset + off, [p0] + [list(d) for d in dims])

    def cE(r, w):          # E grid cell offset
        return r * ROW + w * CELL

    def cD(r, scol):       # ed grid cell offset (dir1 view)
        return r * EROW + scol * ECELL

    with tile.TileContext(nc) as tc:
        with ExitStack() as ctx:
            const = ctx.enter_context(tc.tile_pool(name="const", bufs=1))
            big = ctx.enter_context(tc.tile_pool(name="big", bufs=1))
            slabs = ctx.enter_context(tc.tile_pool(name="slabs", bufs=8))
            scr = ctx.enter_context(tc.tile_pool(name="scr", bufs=2))
            psm = ctx.enter_context(tc.tile_pool(name="psm", bufs=2, space="PSUM"))
            psg = ctx.enter_context(tc.tile_pool(name="psg", bufs=3, space="PSUM"))
            psn = ctx.enter_context(tc.tile_pool(name="psn", bufs=1, space="PSUM"))
            pss = ctx.enter_context(tc.tile_pool(name="pss", bufs=1, space="PSUM"))

            ones_nk = const.tile([128, 1], db)   # lhsT for norm matmuls (K=128, M=1)
            ones_b = const.tile([1, 128], db)    # lhsT for broadcast matmul (K=1, M=128)
            neg2 = const.tile([128, 1], dt)      # ACT bias for exp(2cos - 2)
            ln2 = const.tile([128, 1], dt)       # ACT bias ln(2) for 2/sn
            imask = const.tile([128, 128], db)   # identity mask for gram diag
            nc.vector.memset(ones_nk[:], 1.0)
            nc.vector.memset(ones_b[:], 1.0)
            nc.vector.memset(neg2[:], -2.0)
            nc.vector.memset(ln2[:], LN2)

            # Preload the one ACT table that covers Ln+Exp+Copy so the
            # fixpoint pass never inserts per-transition table reloads
            # (act_func_set_id 6 = natural_log_exp_and_others).
            nc.scalar.add_instruction(mybir.InstLoadActFuncSet(
                name=nc.scalar.bass.get_next_instruction_name(),
                act_func_set_id=6, ins=[], outs=[]))

            s8 = big.tile([128, KC, NSF], d8)            # fp8 raw support
            sT = big.tile([128, KC, NSF], db)            # bf16 scaled support
            ed = big.tile([128, 8 * EROW], db)           # 16KB/p
            E = big.tile([128, 8 * ROW], db)             # 32KB/p DP grid
            nsq = big.tile([128, LG], dt)                # col = l*2+g (|q|^2)
            rqn = big.tile([128, LG], dt)                # 1/|q|
            lnq = big.tile([128, LG], dt)                # ln(nsq) staging
            snsq = big.tile([128, 4], dt)                # |s_f|^2, partition=frame
            slnq = big.tile([128, 4], dt)
            rsn = big.tile([128, 4], db)                 # 2/|s_f| bf16
            rsnT = big.tile([1, 512], db)                # transposed row for bcast
            pbs = big.tile([128, NSF], db)               # bf16 copy of 2/sn bcast
            fin = big.tile([128, 2816], db)              # final-reduction scratch
            junk = big.tile([128, 128], db)              # stt masked-out scratch
            e9m = big.tile([128, 128], dt)               # E9A*E9B fp32
            outf = big.tile([128, 128], dt)              # fp32 logits staging

            # ---------- DMAs upfront, all on sync in dependency order:
            # support + mask first (gate the whole chain), then slabs in l
            # order so descriptors drain in consumption order.
            nc.sync.dma_start(V(s8, 0, [[1, KC * NSF]]), sd.ap())
            nc.sync.dma_start(imask[:], md.ap())
            stiles = {}
            for l in range(L):
                slab = slabs.tile([128, KC, 256], d8)
                stiles[l] = slab
                nc.sync.dma_start(V(slab, 0, [[1, KC * 256]]),
                                  qd.ap()[l * 128:(l + 1) * 128, :])

            # ---------- support prep: fp8 gram norms -> 2/sn -> scale ----------
            # High priority: this chain gates mains(0).  Support norms come
            # from fp8 gram diag blocks (like the query norms) so no squares
            # or [1,512] ln/exp sit on the critical path.
            with tc.high_priority():
                sg = pss.tile([128, 4, 128], dt)
                for b in range(4):
                    for k in range(KC):
                        sb = s8[:, k, b * 128:(b + 1) * 128]
                        nc.tensor.matmul(sg[:, b], sb, sb,
                                         start=(k == 0), stop=(k == KC - 1))
                for b in range(4):
                    nc.vector.scalar_tensor_tensor(
                        junk[:], sg[:, b], 1.0, imask[:],
                        OP.bypass, OP.mult, accum_out=snsq[:, b:b + 1])
                # rsn = 2/|s_f| = exp(-0.5*ln(|s|^2) + ln2), partition=frame
                nc.scalar.activation(slnq[:], snsq[:], ACTF.Ln)
                nc.scalar.activation(rsn[:], slnq[:], ACTF.Exp, bias=ln2[:],
                                     scale=-0.5)
                # transpose each [128,1] column to a [1,128] partition-0 row
                # (PE), gather to one [1,512] row, broadcast into pb via one
                # K=1 matmul, and scale s8 -> bf16 sT in one pass
                pt = psn.tile([1, 4, 128], db, tag="pt")
                for b in range(4):
                    nc.tensor.transpose(pt[:, b], rsn[:, b:b + 1], imask[:])
                nc.vector.tensor_copy(rsnT[:], V(pt, 0, [[1, 512]]))
                pb = psn.tile([128, NSF], dt, tag="pb")
                nc.tensor.matmul(pb[:], ones_b[:], rsnT[:], start=True, stop=True)
                nc.scalar.copy(pbs[:], pb[:])
                for k in range(KC):
                    nc.vector.tensor_tensor(sT[:, k, :], s8[:, k, :], pbs[:],
                                            OP.mult)

            # ---------- query norms: gram matmul + masked diag extract ----------
            gps = {}

            def gram(l):
                gp = psg.tile([128, G, 128], dt, tag="gp")
                gps[l] = gp
                for g in range(G):
                    for k in range(KC):
                        q = stiles[l][:, k, g * 128:(g + 1) * 128]
                        nc.tensor.matmul(gp[:, g], q, q,
                                         start=(k == 0), stop=(k == KC - 1))

            def stt_diag(l):
                # accum_out = sum over free of (gram * I) = |q|^2 per partition
                for g in range(G):
                    lg = l * G + g
                    nc.vector.scalar_tensor_tensor(
                        junk[:], gps[l][:, g], 1.0, imask[:],
                        OP.bypass, OP.mult, accum_out=nsq[:, lg:lg + 1])

            def rqn_batch(lo, hi):
                # rqn = 1/|q| = exp(-0.5*ln(nsq)) on ACT, for lg in [lo, hi)
                nc.scalar.activation(lnq[:, lo:hi], nsq[:, lo:hi], ACTF.Ln)
                nc.scalar.activation(rqn[:, lo:hi], lnq[:, lo:hi], ACTF.Exp,
                                     scale=-0.5)

            def mains(l, split=False):
                # split=True: two N=256 halves with separate exps, so the
                # first ed frames of the row land ~1us earlier (matters only
                # for the last rows, which gate the wavefront tail)
                for g in range(G):
                    lg = l * G + g
                    pm = psm.tile([128, NSF], dt, tag="mm")
                    halves = ((0, 256), (256, 512)) if split else ((0, 512),)
                    for lo, hi in halves:
                        for k in range(KC):
                            nc.tensor.matmul(pm[:, lo:hi],
                                             stiles[l][:, k, g * 128:(g + 1) * 128],
                                             sT[:, k, lo:hi],
                                             start=(k == 0), stop=(k == KC - 1))
                        nsc = (hi - lo) // 64
                        edv = V(ed, cD(l, lo // 64) + g * 64,
                                [[ECELL, nsc], [1, 64]])
                        nc.scalar.activation(
                            edv,
                            pm[:, lo:hi].rearrange("p (s j) -> p s j", s=nsc),
                            ACTF.Exp, bias=neg2[:], scale=rqn[:, lg:lg + 1])

            # All grams + diag extracts + rqn batches run in the head (slabs
            # land well before the scaled support is ready), so the mains own
            # the PE window exclusively and the c-loop is pure wavefront.
            for l in range(L):
                gram(l)
                stt_diag(l)
                rqn_batch(2 * l, 2 * l + 2)
            mains(0)
            mains(1)

            # ---------- DP wavefront (DVE-only, bf16) ----------
            # E cell (r, w) lanes: [dir*128 + g*64 + sj]. The add op has no ed
            # operand so it covers BOTH dirs in one [n, 256] access; the mult
            # splits per dir (ed strides differ: dir2 reads transposed cells).
            def edo(d, r, w):
                return cD(r, w - 1) if d == 0 else cD(w - 1, r)

            def interior(c, lo, hi):
                # batched add + per-dir mults for cells (r, c-1-r), r=lo..hi
                n = hi - lo + 1
                if n < 1:
                    return
                nc.vector.tensor_tensor(
                    V(E, cE(lo, c - lo - 1), [[DIAG, n], [1, 256]]),
                    V(E, cE(lo - 1, c - lo - 2), [[DIAG, n], [1, 256]]),
                    V(E, cE(lo, c - lo - 2), [[DIAG, n], [1, 256]]),
                    OP.add)
                for d in range(2):
                    eds = ED1 if d == 0 else ED2
                    out = V(E, cE(lo, c - lo - 1) + d * 128, [[DIAG, n], [1, 128]])
                    nc.vector.tensor_tensor(
                        out, out,
                        V(ed, edo(d, lo, c - lo), [[eds, n], [1, 128]]),
                        OP.mult)

            for c in range(1, 16):
                if 2 <= c <= 7:
                    mains(c)

                if c == 1:
                    for d in range(2):
                        nc.vector.tensor_copy(
                            V(E, cE(0, 0) + d * 128, [[1, 128]]),
                            V(ed, edo(d, 0, 1), [[1, 128]]))
                elif c <= 8:
                    # row 0: one op covers both dirs; ed dirs are not
                    # adjacent, so a 2-dim AP [dir-delta, 2][1, 128] maps them
                    d2 = edo(1, 0, c) - edo(0, 0, c)
                    nc.vector.tensor_tensor(
                        V(E, cE(0, c - 1), [[1, 256]]),
                        V(E, cE(0, c - 2), [[1, 256]]),
                        V(ed, edo(0, 0, c), [[d2, 2], [1, 128]]), OP.mult)

                if 2 <= c <= 8:
                    for d in range(2):
                        nc.vector.scalar_tensor_tensor(
                            V(E, cE(c - 1, 0) + d * 128, [[1, 128]]),
                            V(E, cE(c - 2, 0) + d * 128, [[1, 128]]), 2.0,
                            V(ed, edo(d, c - 1, 1), [[1, 128]]), OP.add, OP.mult)

                if c <= 8:
                    interior(c, max(1, c - 8), min(7, c - 2))
                elif c == 9:
                    # cells needing only ed rows <= 5 (middle bands of diags
                    # 9-11) run before exp(6)/exp(7) land; the edge bands and
                    # diags 12+ follow in dependency order.
                    for cc in (9, 10, 11):
                        interior(cc, cc - 6, 5)
                if c >= 9:
                    if c <= 11:
                        interior(c, c - 8, c - 7)
                        interior(c, 6, 7)
                    else:
                        interior(c, c - 8, 7)

                # overlap the col-7 pair reductions with the wavefront: cell
                # (r, 7) completes at diag c = r + 8
                if c >= 9 and c % 2 == 1:
                    j = (c - 9) // 2
                    nc.vector.tensor_tensor(
                        V(fin, j * CELL, [[1, CELL]]),
                        V(E, cE(2 * j, 7), [[1, CELL]]),
                        V(E, cE(2 * j + 1, 7), [[1, CELL]]), OP.add)

            # ---------- final pad-column closed form + logits ----------
            # E9 = 2*sum_r E[r][7] - E[7][7], then logits = 0.5*ln(E9A*E9B)
            nc.vector.tensor_tensor(
                V(fin, 2048, [[CELL, 2], [1, CELL]]),
                V(fin, 0, [[2 * CELL, 2], [1, CELL]]),
                V(fin, CELL, [[2 * CELL, 2], [1, CELL]]), OP.add)
            nc.vector.tensor_tensor(
                V(fin, 1024, [[1, CELL]]),
                V(fin, 2048, [[1, CELL]]),
                V(fin, 2048 + CELL, [[1, CELL]]), OP.add)
            e9 = V(fin, 1280, [[1, CELL]])
            nc.vector.scalar_tensor_tensor(
                e9, V(fin, 1024, [[1, CELL]]), 2.0,
                V(E, cE(7, 7), [[1, CELL]]), OP.mult, OP.subtract)
            # e9m = E9A * E9B (fp32), logits = 0.5*ln(e9m)
            nc.vector.tensor_tensor(e9m[:], V(fin, 1280, [[1, 128]]),
                                    V(fin, 1280 + 128, [[1, 128]]), OP.mult)
            outv = outf[:, 0:128]
            nc.scalar.activation(outv, e9m[:], ACTF.Ln)
            nc.vector.tensor_scalar_mul(outv, outv, 0.5)
            # DMA out: logits[q = g*128 + p, sj];  src free f = g*64 + sj
            oap = od.ap()
            dst = bass.AP(oap.tensor, oap.offset, [[NS, 128], [128 * NS, G], [1, NS]])
            nc.sync.dma_start(dst, outv)

    nc.compile()
    return nc


_CACHED = None


def _get_program():
    global _CACHED
    if _CACHED is None:
        _CACHED = _build_program()
    return _CACHED


def _prep_inputs(support_features, query_features):
    """Host-side data movement: shard queries, pad D to 640, reorder layouts."""
    import ml_dtypes
    bf16 = ml_dtypes.bfloat16
    e4m3 = ml_dtypes.float8_e4m3fn
    q = np.ascontiguousarray(query_features, dtype=np.float32)
    s = np.ascontiguousarray(support_features, dtype=np.float32)
    qp = np.zeros((NQ_TOT, L, DPAD), np.float32)
    qp[:, :, :D] = q
    sp = np.zeros((NSF, DPAD), np.float32)
    sp[:, :D] = s.reshape(NSF, D)
    # support frame reorder: scol = s*64 + sj  <->  frame sj*8 + s
    idx = (np.arange(NSF) % NS) * S + (np.arange(NSF) // NS)
    spr = sp[idx]                                   # [512, 640]
    sT_r = np.ascontiguousarray(
        spr.reshape(NSF, KC, 128).transpose(2, 1, 0)).reshape(128, KC * NSF).astype(e4m3)
    im = np.eye(128, dtype=bf16)
    in_maps = []
    for cidx in range(NCORES):
        qs = qp[cidx * NQ:(cidx + 1) * NQ]          # [256, 8, 640]
        q5 = qs.reshape(G, 128, L, KC, 128)          # [g, qi, l, k, dp]
        # slab for l: SBUF [128 part=dp, k, (g,qi)] -> host rows (l, dp), cols (k, g, qi)
        qT_r = np.ascontiguousarray(q5.transpose(2, 4, 3, 0, 1))  # [l, dp, k, g, qi]
        in_maps.append({
            "qslabs": qT_r.reshape(L * 128, KC * 256).astype(e4m3),
            "sfeat8": sT_r,
            "imask": im,
        })
    return in_maps


def kernel(support_features, query_features):
    from concourse.bass_utils import run_bass_kernel_spmd
    nc = _get_program()
    in_maps = _prep_inputs(support_features, query_features)
    res = run_bass_kernel_spmd(nc, in_maps, list(range(NCORES)))
    out = np.concatenate([res.results[i]["logits"] for i in range(NCORES)], axis=0)
    return out.astype(np.float32)
